# revision 50
# baseline (speedup 1.0000x reference)
"""BifurcationAttention TRN2 kernel.

Full-input contract: kernel(**inputs) takes the unsharded inputs and returns
the full [B, S, D] output. Internally shards across 8 NeuronCores:
core i handles batch i//4 and heads {2*(i%4), 2*(i%4)+1}.

Math per (b, h):
  q = x[b] @ Wq[h].T + bq[h]          # [S, HD]
  k = (x[b] @ Wk[h].T + bk[h]) / 8    # scale folded into Wk/bk host-side
  v = x[b] @ Wv[h].T + bv[h]
  sT = k @ q.T + coeff*noise[b,h].T   # [k, q] transposed-score layout
  aT = exp(sT)                        # no max subtraction (scores O(1))
  ctxT_aug = [v | 1].T @ aT           # [HD+1, q]; row HD = softmax denom
  out[b, :, h*HD:(h+1)*HD] = (ctxT_aug[:HD] / ctxT_aug[HD]).T  # on host

Device pipeline per core: f32r (fast-fp32) matmuls on the PE for the QKV
projections / scores / context; the fp16 noise add runs on the VectorE
(PSUM + SBUF -> SBUF); exp runs on ScalarE as one [128, 2048] activation
per k-tile; context accumulates over all 16 k-tiles in 4 PSUM banks with a
ones-column producing the softmax denominator for free. The kernel returns
the raw [2, HD+1, S] accumulator per core; the final divide + transpose
(0.5MB/core) happens on the host in _assemble.
"""

import numpy as np

B, S, D = 2, 2048, 512
H = 8
HD = D // H  # 64
THRESHOLD = 0.5
N_CORES = 8
P = 128  # partitions
NK = S // P  # 16 k-token tiles
NQC = S // 512  # 4 q column chunks of 512
NFC = D // P  # 4 feature chunks

_cache = {}


def _compute_coeff(bif_param: np.float32) -> np.float32:
    # matches reference: r = sigmoid(bif); f = r*sin(pi*r); 0.05 if |f-0.5|<0.1
    r = np.float32(1.0) / (np.float32(1.0) + np.exp(-np.float32(bif_param)))
    bf = r * np.sin(np.float32(np.pi) * r)
    return np.float32(0.05) if abs(float(bf) - THRESHOLD) < 0.1 else np.float32(0.0)


def _get_noise_t() -> np.ndarray:
    """noise[b,h].T as [B, H, S(k), S(q)] float32 (unscaled), matching
    jax.random.normal(key(42), [B,H,S,S], f32) from the reference."""
    if "noise_t" not in _cache:
        import jax

        # Must be generated exactly as the reference does — on the ambient
        # default jax backend (RNG bits differ between cpu and axon backends).
        noise = np.asarray(
            jax.random.normal(
                jax.random.key(42), (B, H, S, S), dtype=jax.numpy.float32
            )
        )
        _cache["noise_t"] = np.ascontiguousarray(np.swapaxes(noise, -1, -2))
    return _cache["noise_t"]


def _build_nc():
    """Build + compile the single-core Bass program (SPMD across 8 cores)."""
    if "nc" in _cache:
        return _cache["nc"]

    from contextlib import ExitStack

    import concourse.bacc as bacc
    import concourse.mybir as mybir
    import concourse.tile as tile
    from concourse.masks import make_identity

    f32 = mybir.dt.float32
    f32r = mybir.dt.float32r
    f16 = mybir.dt.float16
    AF = mybir.ActivationFunctionType

    nc = bacc.Bacc("TRN2", target_bir_lowering=False, debug=False)

    # host-prepped layouts: xt as [128, NFC, S], weights as [128, NFC, 128]
    xt_d = nc.dram_tensor("xt", [P, NFC, S], f32r, kind="ExternalInput").ap()
    wqt_d = nc.dram_tensor("wqt", [P, NFC, P], f32r, kind="ExternalInput").ap()
    wkt_d = nc.dram_tensor("wkt", [P, NFC, P], f32r, kind="ExternalInput").ap()
    wvt_d = nc.dram_tensor("wvt", [P, NFC, P], f32r, kind="ExternalInput").ap()
    bq_d = nc.dram_tensor("bq", [P, 1], f32, kind="ExternalInput").ap()
    bk_d = nc.dram_tensor("bk", [P, 1], f32, kind="ExternalInput").ap()
    bv_d = nc.dram_tensor("bv", [P, 1], f32, kind="ExternalInput").ap()
    noi_d = nc.dram_tensor("noi", [2, S, S], f16, kind="ExternalInput").ap()
    ctx_d = nc.dram_tensor("ctx", [2, HD + 1, S], f32, kind="ExternalOutput").ap()

    with tile.TileContext(nc) as tc, ExitStack() as ctx:
        const = ctx.enter_context(tc.tile_pool(name="const", bufs=1))
        big = ctx.enter_context(tc.tile_pool(name="big", bufs=1))
        attn = ctx.enter_context(tc.tile_pool(name="attn", bufs=8))
        sump = ctx.enter_context(tc.tile_pool(name="sump", bufs=5))
        nzp = ctx.enter_context(tc.tile_pool(name="nzp", bufs=8))
        ctop = ctx.enter_context(tc.tile_pool(name="ctop", bufs=2))
        psA = ctx.enter_context(tc.tile_pool(name="psA", bufs=2, space="PSUM"))
        psB = ctx.enter_context(tc.tile_pool(name="psB", bufs=4, space="PSUM"))

        # ---- constants ----
        idf = const.tile([P, P], f32, tag="idf")
        make_identity(nc, idf)

        xt_sb = const.tile([P, NFC, S], f32r, tag="xt")
        w_sb = {}
        b_sb = {}
        for name in ("q", "k", "v"):
            w_sb[name] = const.tile(
                [P, NFC, P], f32r, tag=f"w{name}", name=f"w{name}_sb"
            )
            b_sb[name] = const.tile([P, 1], f32, tag=f"b{name}", name=f"b{name}_sb")

        # smallest-first DMA order so the first projection group starts ASAP
        nc.sync.dma_start(out=w_sb["q"], in_=wqt_d)
        nc.sync.dma_start(out=b_sb["q"], in_=bq_d)
        for qq in range(4):
            nc.sync.dma_start(
                out=xt_sb[:, :, qq * 512 : (qq + 1) * 512],
                in_=xt_d[:, :, qq * 512 : (qq + 1) * 512],
            )
            if qq == 1:
                nc.sync.dma_start(out=w_sb["k"], in_=wkt_d)
                nc.sync.dma_start(out=b_sb["k"], in_=bk_d)
            elif qq == 2:
                nc.sync.dma_start(out=w_sb["v"], in_=wvt_d)
                nc.sync.dma_start(out=b_sb["v"], in_=bv_d)

        # warm the PE HAM clock gate with tiny matmuls while inputs stream in
        warm = psA.tile([P, 8], f32, tag="psA", name="warm")
        for _ in range(112):
            nc.tensor.matmul(warm, lhsT=idf, rhs=idf[:, 0:8], start=True, stop=True)

        # ---- projections [128 (2 heads x 64), 2048 tokens] ----
        proj_sb = {}
        for name in ("q", "k", "v"):
            pdt = f32 if name == "v" else f32r
            proj_sb[name] = big.tile([P, S], pdt, tag=f"t{name}", name=f"t{name}_sb")

        def emit_proj(name, qq, evac_engine="dve"):
            ps = psB.tile([P, 512], f32, tag="psB", name="ps_proj")
            for fc in range(NFC):
                nc.tensor.matmul(
                    ps,
                    lhsT=w_sb[name][:, fc, :],
                    rhs=xt_sb[:, fc, qq * 512 : (qq + 1) * 512],
                    start=(fc == 0),
                    stop=(fc == NFC - 1),
                )
            if evac_engine == "dve":
                nc.vector.tensor_scalar_add(
                    proj_sb[name][:, qq * 512 : (qq + 1) * 512],
                    ps,
                    b_sb[name],
                )
            else:
                nc.scalar.activation(
                    proj_sb[name][:, qq * 512 : (qq + 1) * 512],
                    ps,
                    AF.Identity,
                    bias=b_sb[name],
                )

        # pre-loop: just enough projection for the first score half-tiles;
        # everything else drip-feeds into the head-0 loop with ACT evacs so
        # the DVE add stream is never interrupted
        emit_proj("q", 0)
        emit_proj("q", 1)
        emit_proj("k", 0)

        v_sb = big.tile([P, 2, NK, HD + 1], f32r, tag="vaug")
        ones32 = const.tile([P, 2 * NK], f32, tag="ones32")
        nc.vector.memset(ones32, 1.0)
        nc.vector.tensor_copy(
            v_sb[:, :, :, HD : HD + 1],
            ones32.rearrange("p (a b c) -> p a b c", b=NK, c=1),
        )

        def emit_vt(hh, g):
            ps = psB.tile([P, 512], f32, tag="psB", name="ps_vt")
            for t8 in range(8):
                tt = g * 8 + t8
                nc.tensor.transpose(
                    ps[:, t8 * HD : (t8 + 1) * HD],
                    proj_sb["v"][hh * HD : (hh + 1) * HD, tt * P : (tt + 1) * P],
                    idf[hh * HD : (hh + 1) * HD, hh * HD : (hh + 1) * HD],
                )
            nc.scalar.activation(
                v_sb[:, hh, g * 8 : (g + 1) * 8, 0:HD],
                ps.rearrange("p (t d) -> p t d", t=8),
                AF.Copy,
            )

        # (emitter, args) items drip-fed one per head-0 half-tile (q2+q3
        # jointly first: the very next half needs both)
        defer = [
            [("p", "q", 2), ("p", "q", 3)],
            [("p", "k", 1)],
            [("p", "k", 2)],
            [("p", "k", 3)],
            [("p", "v", 0)],
            [("p", "v", 1)],
            [("p", "v", 2)],
            [("p", "v", 3)],
            [("t", 0, 0)],
            [("t", 0, 1)],
            [("t", 1, 0)],
            [("t", 1, 1)],
        ]

        def emit_deferred(slot):
            if slot < len(defer):
                for item in defer[slot]:
                    if item[0] == "p":
                        emit_proj(item[1], item[2], evac_engine="act")
                    else:
                        emit_vt(item[1], item[2])

        # ---- attention main loop ----
        # scores on PE -> noise add on DVE (into SBUF) -> exp on ACT
        # -> ctx matmuls on PE several kt behind (never waiting on exp)
        for hh in range(2):
            hs = slice(hh * HD, (hh + 1) * HD)
            depth = 6 if hh == 0 else 1
            C = None  # allocated lazily, after the drip-fed psB users
            pending = []

            def emit_ctx(item):
                nonlocal C
                if C is None:
                    C = [
                        psB.tile([P, 512], f32, tag="psB", name=f"C{hh}_{qc}")
                        for qc in range(NQC)
                    ]
                pkt, pat = item
                for qc in range(NQC):
                    nc.tensor.matmul(
                        C[qc][0 : HD + 1, :],
                        lhsT=v_sb[:, hh, pkt, :],
                        rhs=pat[:, qc * 512 : (qc + 1) * 512],
                        start=(pkt == 0),
                        stop=(pkt == NK - 1),
                    )

            for kt in range(NK):
                ks = slice(kt * P, (kt + 1) * P)
                at = attn.tile([P, S], f32r, tag="at")
                sum_t = sump.tile([P, S], f32, tag="sum")
                for half in range(2):
                    nz = nzp.tile([P, 1024], f16, tag="nz")
                    nc.sync.dma_start(
                        out=nz,
                        in_=noi_d[hh, ks, half * 1024 : (half + 1) * 1024],
                    )
                    ps = psA.tile([P, 1024], f32, tag="psA")
                    for qc2 in range(2):
                        q0 = half * 1024 + qc2 * 512
                        nc.tensor.matmul(
                            ps[:, qc2 * 512 : (qc2 + 1) * 512],
                            lhsT=proj_sb["k"][hs, ks],
                            rhs=proj_sb["q"][hs, q0 : q0 + 512],
                            start=True,
                            stop=True,
                        )
                    nc.vector.tensor_add(
                        sum_t[:, half * 1024 : (half + 1) * 1024], ps, nz
                    )
                    if hh == 0:
                        if kt < 3:
                            # lead-in: exp per half so the left-column pipeline
                            # runs while the last x-quarters are still loading
                            nc.scalar.activation(
                                at[:, half * 1024 : (half + 1) * 1024],
                                sum_t[:, half * 1024 : (half + 1) * 1024],
                                AF.Exp,
                            )
                        emit_deferred(2 * kt + half)
                if hh == 0 and kt < 3:
                    pass
                elif kt == NK - 1:
                    # split the last exp so the final ctx matmuls start early
                    nc.scalar.activation(at[:, 0:1024], sum_t[:, 0:1024], AF.Exp)
                    nc.scalar.activation(
                        at[:, 1024:2048], sum_t[:, 1024:2048], AF.Exp
                    )
                else:
                    nc.scalar.activation(at, sum_t, AF.Exp)
                pending.append((kt, at))
                if len(pending) > depth:
                    emit_ctx(pending.pop(0))
            for item in pending:
                emit_ctx(item)

            # ---- evacuate raw [ctxT | denom] to DRAM via SBUF; the host
            # does the tiny divide+transpose. Split across DVE and ACT ----
            assert C is not None
            cto = ctop.tile([HD + 1, S], f32, tag="cto")
            for qc in range(NQC):
                if qc % 2 == 0:
                    nc.vector.tensor_copy(
                        cto[:, qc * 512 : (qc + 1) * 512], C[qc][0 : HD + 1, :]
                    )
                else:
                    nc.scalar.activation(
                        cto[:, qc * 512 : (qc + 1) * 512],
                        C[qc][0 : HD + 1, :],
                        AF.Copy,
                    )
                if hh == 1:
                    # last head: store per-chunk so the DMA overlaps the
                    # remaining evacuations instead of sitting on the tail
                    nc.sync.dma_start(
                        out=ctx_d[hh, :, qc * 512 : (qc + 1) * 512],
                        in_=cto[:, qc * 512 : (qc + 1) * 512],
                    )
            if hh == 0:
                nc.sync.dma_start(out=ctx_d[hh, :, :], in_=cto)

    nc.compile()
    _cache["nc"] = nc
    return nc


def _make_in_maps(inputs: dict) -> list[dict]:
    x = np.asarray(inputs["x"], dtype=np.float32)
    Wq = np.asarray(inputs["Wq"], dtype=np.float32)
    Wk = np.asarray(inputs["Wk"], dtype=np.float32)
    Wv = np.asarray(inputs["Wv"], dtype=np.float32)
    bq = np.asarray(inputs["bq"], dtype=np.float32)
    bk = np.asarray(inputs["bk"], dtype=np.float32)
    bv = np.asarray(inputs["bv"], dtype=np.float32)
    coeff = _compute_coeff(np.float32(np.asarray(inputs["bif_param"])))
    scale = np.float32(1.0 / np.sqrt(np.float32(HD)))

    noise_t = _get_noise_t()

    in_maps = []
    for core in range(N_CORES):
        b = core // 4
        p = core % 4
        h0 = 2 * p
        rows = slice(h0 * HD, h0 * HD + P)  # 128 output dims = 2 heads
        noi = np.empty((2, S, S), dtype=np.float16)
        noi[0] = (coeff * noise_t[b, h0]).astype(np.float16)
        noi[1] = (coeff * noise_t[b, h0 + 1]).astype(np.float16)
        def chunk(a):  # [D, M] -> [128, NFC, M]
            return np.ascontiguousarray(
                a.reshape(NFC, P, a.shape[1]).transpose(1, 0, 2)
            )

        in_maps.append(
            {
                "xt": chunk(x[b].T),
                "wqt": chunk(Wq[rows].T),
                "wkt": chunk(Wk[rows].T * scale),
                "wvt": chunk(Wv[rows].T),
                "bq": np.ascontiguousarray(bq[rows].reshape(P, 1)),
                "bk": np.ascontiguousarray(bk[rows].reshape(P, 1) * scale),
                "bv": np.ascontiguousarray(bv[rows].reshape(P, 1)),
                "noi": noi,
            }
        )
    return in_maps


def _assemble(results: list[dict]) -> np.ndarray:
    out = np.empty((B, S, D), dtype=np.float32)
    for core in range(N_CORES):
        b = core // 4
        p = core % 4
        raw = results[core]["ctx"]  # [2, HD+1, S]
        for hh in range(2):
            ctx_t = raw[hh, :HD, :] / raw[hh, HD, :]
            out[b, :, (2 * p + hh) * HD : (2 * p + hh + 1) * HD] = ctx_t.T
    return out


def kernel(**inputs) -> np.ndarray:
    from concourse.bass_utils import run_bass_kernel_spmd

    nc = _build_nc()
    in_maps = _make_in_maps(inputs)
    res = run_bass_kernel_spmd(nc, in_maps, core_ids=list(range(N_CORES)))
    return _assemble(res.results)


# revision 51
# speedup vs baseline: 1.0125x; 1.0125x over previous
"""BifurcationAttention TRN2 kernel.

Full-input contract: kernel(**inputs) takes the unsharded inputs and returns
the full [B, S, D] output. Internally shards across 8 NeuronCores:
core i handles batch i//4 and heads {2*(i%4), 2*(i%4)+1}.

Math per (b, h):
  q = x[b] @ Wq[h].T + bq[h]          # [S, HD]
  k = (x[b] @ Wk[h].T + bk[h]) / 8    # scale folded into Wk/bk host-side
  v = x[b] @ Wv[h].T + bv[h]
  sT = k @ q.T + coeff*noise[b,h].T   # [k, q] transposed-score layout
  aT = exp(sT)                        # no max subtraction (scores O(1))
  ctxT_aug = [v | 1].T @ aT           # [HD+1, q]; row HD = softmax denom
  out[b, :, h*HD:(h+1)*HD] = (ctxT_aug[:HD] / ctxT_aug[HD]).T  # on host

Device pipeline per core: f32r (fast-fp32) matmuls on the PE for the QKV
projections / scores / context; the fp16 noise add runs on the VectorE
(PSUM + SBUF -> SBUF); exp runs on ScalarE as one [128, 2048] activation
per k-tile; context accumulates over all 16 k-tiles in 4 PSUM banks with a
ones-column producing the softmax denominator for free. The kernel returns
the raw [2, HD+1, S] accumulator per core; the final divide + transpose
(0.5MB/core) happens on the host in _assemble.
"""

import numpy as np

B, S, D = 2, 2048, 512
H = 8
HD = D // H  # 64
THRESHOLD = 0.5
N_CORES = 8
P = 128  # partitions
NK = S // P  # 16 k-token tiles
NQC = S // 512  # 4 q column chunks of 512
NFC = D // P  # 4 feature chunks

_cache = {}


def _compute_coeff(bif_param: np.float32) -> np.float32:
    # matches reference: r = sigmoid(bif); f = r*sin(pi*r); 0.05 if |f-0.5|<0.1
    r = np.float32(1.0) / (np.float32(1.0) + np.exp(-np.float32(bif_param)))
    bf = r * np.sin(np.float32(np.pi) * r)
    return np.float32(0.05) if abs(float(bf) - THRESHOLD) < 0.1 else np.float32(0.0)


def _get_noise_t() -> np.ndarray:
    """noise[b,h].T as [B, H, S(k), S(q)] float32 (unscaled), matching
    jax.random.normal(key(42), [B,H,S,S], f32) from the reference."""
    if "noise_t" not in _cache:
        import jax

        # Must be generated exactly as the reference does — on the ambient
        # default jax backend (RNG bits differ between cpu and axon backends).
        noise = np.asarray(
            jax.random.normal(
                jax.random.key(42), (B, H, S, S), dtype=jax.numpy.float32
            )
        )
        _cache["noise_t"] = np.ascontiguousarray(np.swapaxes(noise, -1, -2))
    return _cache["noise_t"]


def _build_nc():
    """Build + compile the single-core Bass program (SPMD across 8 cores)."""
    if "nc" in _cache:
        return _cache["nc"]

    from contextlib import ExitStack

    import concourse.bacc as bacc
    import concourse.mybir as mybir
    import concourse.tile as tile
    from concourse.masks import make_identity

    f32 = mybir.dt.float32
    f32r = mybir.dt.float32r
    f16 = mybir.dt.float16
    AF = mybir.ActivationFunctionType

    nc = bacc.Bacc("TRN2", target_bir_lowering=False, debug=False)

    # host-prepped layouts: xt as [128, NFC, S], weights as [128, NFC, 128]
    xt_d = nc.dram_tensor("xt", [P, NFC, S], f32r, kind="ExternalInput").ap()
    wqt_d = nc.dram_tensor("wqt", [P, NFC, P], f32r, kind="ExternalInput").ap()
    wkt_d = nc.dram_tensor("wkt", [P, NFC, P], f32r, kind="ExternalInput").ap()
    wvt_d = nc.dram_tensor("wvt", [P, NFC, P], f32r, kind="ExternalInput").ap()
    bq_d = nc.dram_tensor("bq", [P, 1], f32, kind="ExternalInput").ap()
    bk_d = nc.dram_tensor("bk", [P, 1], f32, kind="ExternalInput").ap()
    bv_d = nc.dram_tensor("bv", [P, 1], f32, kind="ExternalInput").ap()
    noi_d = nc.dram_tensor("noi", [2, S, S], f16, kind="ExternalInput").ap()
    ctx_d = nc.dram_tensor("ctx", [2, HD + 1, S], f32, kind="ExternalOutput").ap()

    with tile.TileContext(nc) as tc, ExitStack() as ctx:
        const = ctx.enter_context(tc.tile_pool(name="const", bufs=1))
        big = ctx.enter_context(tc.tile_pool(name="big", bufs=1))
        attn = ctx.enter_context(tc.tile_pool(name="attn", bufs=8))
        sump = ctx.enter_context(tc.tile_pool(name="sump", bufs=5))
        nzp = ctx.enter_context(tc.tile_pool(name="nzp", bufs=8))
        ctop = ctx.enter_context(tc.tile_pool(name="ctop", bufs=2))
        psA = ctx.enter_context(tc.tile_pool(name="psA", bufs=2, space="PSUM"))
        psB = ctx.enter_context(tc.tile_pool(name="psB", bufs=4, space="PSUM"))

        # ---- constants ----
        idf = const.tile([P, P], f32, tag="idf")
        make_identity(nc, idf)

        xt_sb = const.tile([P, NFC, S], f32r, tag="xt")
        w_sb = {}
        b_sb = {}
        for name in ("q", "k", "v"):
            w_sb[name] = const.tile(
                [P, NFC, P], f32r, tag=f"w{name}", name=f"w{name}_sb"
            )
            b_sb[name] = const.tile([P, 1], f32, tag=f"b{name}", name=f"b{name}_sb")

        # smallest-first DMA order so the first projection group starts ASAP
        nc.sync.dma_start(out=w_sb["q"], in_=wqt_d)
        nc.sync.dma_start(out=b_sb["q"], in_=bq_d)
        for qq in range(4):
            nc.sync.dma_start(
                out=xt_sb[:, :, qq * 512 : (qq + 1) * 512],
                in_=xt_d[:, :, qq * 512 : (qq + 1) * 512],
            )
            if qq == 1:
                nc.sync.dma_start(out=w_sb["k"], in_=wkt_d)
                nc.sync.dma_start(out=b_sb["k"], in_=bk_d)
            elif qq == 2:
                nc.sync.dma_start(out=w_sb["v"], in_=wvt_d)
                nc.sync.dma_start(out=b_sb["v"], in_=bv_d)

        # warm the PE HAM clock gate with tiny matmuls while inputs stream in
        warm = psA.tile([P, 8], f32, tag="psA", name="warm")
        for _ in range(112):
            nc.tensor.matmul(warm, lhsT=idf, rhs=idf[:, 0:8], start=True, stop=True)

        # ---- projections [128 (2 heads x 64), 2048 tokens] ----
        proj_sb = {}
        for name in ("q", "k", "v"):
            pdt = f32 if name == "v" else f32r
            proj_sb[name] = big.tile([P, S], pdt, tag=f"t{name}", name=f"t{name}_sb")

        def emit_proj(name, qq, evac_engine="dve"):
            ps = psB.tile([P, 512], f32, tag="psB", name="ps_proj")
            for fc in range(NFC):
                nc.tensor.matmul(
                    ps,
                    lhsT=w_sb[name][:, fc, :],
                    rhs=xt_sb[:, fc, qq * 512 : (qq + 1) * 512],
                    start=(fc == 0),
                    stop=(fc == NFC - 1),
                )
            if evac_engine == "dve":
                nc.vector.tensor_scalar_add(
                    proj_sb[name][:, qq * 512 : (qq + 1) * 512],
                    ps,
                    b_sb[name],
                )
            else:
                nc.scalar.activation(
                    proj_sb[name][:, qq * 512 : (qq + 1) * 512],
                    ps,
                    AF.Identity,
                    bias=b_sb[name],
                )

        # pre-loop: just enough projection for the first score half-tiles;
        # everything else drip-feeds into the head-0 loop with ACT evacs so
        # the DVE add stream is never interrupted
        emit_proj("q", 0)
        emit_proj("q", 1)
        emit_proj("k", 0)

        v_sb = big.tile([P, 2, NK, HD + 1], f32r, tag="vaug")
        ones32 = const.tile([P, 2 * NK], f32, tag="ones32")
        nc.vector.memset(ones32, 1.0)
        nc.vector.tensor_copy(
            v_sb[:, :, :, HD : HD + 1],
            ones32.rearrange("p (a b c) -> p a b c", b=NK, c=1),
        )

        def emit_vt(hh, g):
            ps = psB.tile([P, 512], f32, tag="psB", name="ps_vt")
            for t8 in range(8):
                tt = g * 8 + t8
                nc.tensor.transpose(
                    ps[:, t8 * HD : (t8 + 1) * HD],
                    proj_sb["v"][hh * HD : (hh + 1) * HD, tt * P : (tt + 1) * P],
                    idf[hh * HD : (hh + 1) * HD, hh * HD : (hh + 1) * HD],
                )
            nc.scalar.activation(
                v_sb[:, hh, g * 8 : (g + 1) * 8, 0:HD],
                ps.rearrange("p (t d) -> p t d", t=8),
                AF.Copy,
            )

        # (emitter, args) items drip-fed one per head-0 half-tile (q2+q3
        # jointly first: the very next half needs both)
        defer = [
            [("p", "q", 2), ("p", "q", 3)],
            [("p", "k", 1)],
            [("p", "k", 2)],
            [("p", "k", 3)],
            [("p", "v", 0)],
            [("p", "v", 1)],
            [("p", "v", 2)],
            [("p", "v", 3)],
            [("t", 0, 0)],
            [("t", 0, 1)],
            [("t", 1, 0)],
            [("t", 1, 1)],
        ]

        def emit_deferred(slot):
            if slot < len(defer):
                for item in defer[slot]:
                    if item[0] == "p":
                        emit_proj(item[1], item[2], evac_engine="act")
                    else:
                        emit_vt(item[1], item[2])

        # ---- attention main loop ----
        # scores on PE -> noise add on DVE (into SBUF) -> exp on ACT
        # -> ctx matmuls on PE several kt behind (never waiting on exp)
        for hh in range(2):
            hs = slice(hh * HD, (hh + 1) * HD)
            depth = 6 if hh == 0 else 1
            C = None  # allocated lazily, after the drip-fed psB users
            pending = []

            def emit_ctx(item):
                nonlocal C
                if C is None:
                    C = [
                        psB.tile([P, 512], f32, tag="psB", name=f"C{hh}_{qc}")
                        for qc in range(NQC)
                    ]
                pkt, pat = item
                for qc in range(NQC):
                    nc.tensor.matmul(
                        C[qc][0 : HD + 1, :],
                        lhsT=v_sb[:, hh, pkt, :],
                        rhs=pat[:, qc * 512 : (qc + 1) * 512],
                        start=(pkt == 0),
                        stop=(pkt == NK - 1),
                    )

            for kt in range(NK):
                ks = slice(kt * P, (kt + 1) * P)
                at = attn.tile([P, S], f32r, tag="at")
                sum_t = sump.tile([P, S], f32, tag="sum")
                for half in range(2):
                    nz = nzp.tile([P, 1024], f16, tag="nz")
                    nc.sync.dma_start(
                        out=nz,
                        in_=noi_d[hh, ks, half * 1024 : (half + 1) * 1024],
                    )
                    ps = psA.tile([P, 1024], f32, tag="psA")
                    for qc2 in range(2):
                        q0 = half * 1024 + qc2 * 512
                        nc.tensor.matmul(
                            ps[:, qc2 * 512 : (qc2 + 1) * 512],
                            lhsT=proj_sb["k"][hs, ks],
                            rhs=proj_sb["q"][hs, q0 : q0 + 512],
                            start=True,
                            stop=True,
                        )
                    nc.vector.tensor_add(
                        sum_t[:, half * 1024 : (half + 1) * 1024], ps, nz
                    )
                    if hh == 0:
                        if kt < 3:
                            # lead-in: exp per half so the left-column pipeline
                            # runs while the last x-quarters are still loading
                            nc.scalar.activation(
                                at[:, half * 1024 : (half + 1) * 1024],
                                sum_t[:, half * 1024 : (half + 1) * 1024],
                                AF.Exp,
                            )
                        emit_deferred(2 * kt + half)
                if hh == 0 and kt < 3:
                    pass
                elif kt == NK - 1:
                    # split the last exp so the final ctx matmuls start early
                    nc.scalar.activation(at[:, 0:1024], sum_t[:, 0:1024], AF.Exp)
                    nc.scalar.activation(
                        at[:, 1024:2048], sum_t[:, 1024:2048], AF.Exp
                    )
                else:
                    nc.scalar.activation(at, sum_t, AF.Exp)
                pending.append((kt, at))
                if len(pending) > depth:
                    emit_ctx(pending.pop(0))
            for item in pending:
                emit_ctx(item)

            # ---- evacuate raw [ctxT | denom] to DRAM via SBUF; the host
            # does the tiny divide+transpose. Split across DVE and ACT ----
            assert C is not None
            cto = ctop.tile([HD + 1, S], f32, tag="cto")
            for qc in range(NQC):
                # head 0 evacuates on ACT only: its copies land mid-stream
                # during head 1's noise-adds and would interrupt the DVE pole
                if hh == 1 and qc % 2 == 0:
                    nc.vector.tensor_copy(
                        cto[:, qc * 512 : (qc + 1) * 512], C[qc][0 : HD + 1, :]
                    )
                else:
                    nc.scalar.activation(
                        cto[:, qc * 512 : (qc + 1) * 512],
                        C[qc][0 : HD + 1, :],
                        AF.Copy,
                    )
                if hh == 1:
                    # last head: store per-chunk so the DMA overlaps the
                    # remaining evacuations instead of sitting on the tail
                    nc.sync.dma_start(
                        out=ctx_d[hh, :, qc * 512 : (qc + 1) * 512],
                        in_=cto[:, qc * 512 : (qc + 1) * 512],
                    )
            if hh == 0:
                nc.sync.dma_start(out=ctx_d[hh, :, :], in_=cto)

    nc.compile()
    _cache["nc"] = nc
    return nc


def _make_in_maps(inputs: dict) -> list[dict]:
    x = np.asarray(inputs["x"], dtype=np.float32)
    Wq = np.asarray(inputs["Wq"], dtype=np.float32)
    Wk = np.asarray(inputs["Wk"], dtype=np.float32)
    Wv = np.asarray(inputs["Wv"], dtype=np.float32)
    bq = np.asarray(inputs["bq"], dtype=np.float32)
    bk = np.asarray(inputs["bk"], dtype=np.float32)
    bv = np.asarray(inputs["bv"], dtype=np.float32)
    coeff = _compute_coeff(np.float32(np.asarray(inputs["bif_param"])))
    scale = np.float32(1.0 / np.sqrt(np.float32(HD)))

    noise_t = _get_noise_t()

    in_maps = []
    for core in range(N_CORES):
        b = core // 4
        p = core % 4
        h0 = 2 * p
        rows = slice(h0 * HD, h0 * HD + P)  # 128 output dims = 2 heads
        noi = np.empty((2, S, S), dtype=np.float16)
        noi[0] = (coeff * noise_t[b, h0]).astype(np.float16)
        noi[1] = (coeff * noise_t[b, h0 + 1]).astype(np.float16)
        def chunk(a):  # [D, M] -> [128, NFC, M]
            return np.ascontiguousarray(
                a.reshape(NFC, P, a.shape[1]).transpose(1, 0, 2)
            )

        in_maps.append(
            {
                "xt": chunk(x[b].T),
                "wqt": chunk(Wq[rows].T),
                "wkt": chunk(Wk[rows].T * scale),
                "wvt": chunk(Wv[rows].T),
                "bq": np.ascontiguousarray(bq[rows].reshape(P, 1)),
                "bk": np.ascontiguousarray(bk[rows].reshape(P, 1) * scale),
                "bv": np.ascontiguousarray(bv[rows].reshape(P, 1)),
                "noi": noi,
            }
        )
    return in_maps


def _assemble(results: list[dict]) -> np.ndarray:
    out = np.empty((B, S, D), dtype=np.float32)
    for core in range(N_CORES):
        b = core // 4
        p = core % 4
        raw = results[core]["ctx"]  # [2, HD+1, S]
        for hh in range(2):
            ctx_t = raw[hh, :HD, :] / raw[hh, HD, :]
            out[b, :, (2 * p + hh) * HD : (2 * p + hh + 1) * HD] = ctx_t.T
    return out


def kernel(**inputs) -> np.ndarray:
    from concourse.bass_utils import run_bass_kernel_spmd

    nc = _build_nc()
    in_maps = _make_in_maps(inputs)
    res = run_bass_kernel_spmd(nc, in_maps, core_ids=list(range(N_CORES)))
    return _assemble(res.results)
